# revision 8
# baseline (speedup 1.0000x reference)
"""STBlock (temporal attn -> spatial attn -> ChebConv + residual, relu) on 8 trn2 cores.

Sharding: data-parallel over batch B=8, one batch element per core.

v2 design: t-major feature layout d' = t*32 + f everywhere, zero PE transposes.
  - host uploads x twice in bf16: natural f-major (score_t) and pre-transposed
    t-major (768, 1024) for XT.
  - temporal attention applied in transposed space: TT' = (E_att^T (x) I32) @ XT
    via an on-device Kronecker-expanded E'' (built with 6 tiny matmuls + 36
    broadcast multiplies).
  - all on-device transposes (TN, SAT, SNT, P1T) are DMA xbar block transposes
    (bf16 128x128) on otherwise-idle DMA engines.
  - softmax skips max-subtraction (logits bounded by |Vs| row sums) and defers
    normalization into the SN psum evacuation scale (per-partition 1/sum).
  - final Cheb+residual projection consumes 64-row (2t x 32f) strips of the
    t-major transposed tensors against block-diag weights: 96 K=64 matmuls,
    output produced transposed (1536, 1024) and un-transposed on host.

Per-core dataflow:
  XN[8]  (128n, 768=(t,f)) bf16   <- dma xb16  (f-major natural, score_t only)
  XT[6]  (128d', 1024n) bf16      <- dma xb16T (host-transposed t-major)
  score_t (24,24) = 256 bf16 MMs; E_att = softmax(Ve sigmoid(score_t) + be)
  E2[6]  (128, 768) bf16 = E'' chunks (Kron expand of eatt via REP-matmuls + P32)
  TT[6]  (128d', 1024n) bf16 = sum_p E2[p,q].T @ XT[p]   (72 MMs)
  TN[8]  (128n, 768d') bf16 <- 48 dma transposes of TT
  SG[8]  (128, 1024) bf16 = sigmoid(TT.T TT)             (96 MMs)
  eexp_i (128, 1024) bf16 = exp(Vs@SG + bs), row sums -> sinv[i] (128 MMs)
  SAT[8] (128m, 1024n) bf16 <- 64 dma transposes of eexp
  SN[8]  (128n, 768) bf16 = sinv * SAT.T @ TN            (128 MMs)
  SNT[6] <- 48 dma transposes; P1[8] = LT.T @ SN (128 MMs); P1T[6] <- 48 dma T
  P2T[6] = 2*(P1.T@LT per chunk) - SNT                   (96 MMs)
  OUTT chunks c=0..11 (128=(2t,64g), 1024n): 4 accumulated K=64 MMs per half
    vs block-diag W4 (SNT,P1T,P2T,XT) -> relu(+bias) -> dma (1536,1024) fp32
Host un-transposes the output.
"""
import numpy as np

B, N, F, T, G = 8, 1024, 32, 24, 64
D = F * T            # 768
NCH = N // 128       # 8 n-chunks
DCH = D // 128       # 6 d-chunks
GT = G * T           # 1536

_compiled = {}


def _build():
    if "nc" in _compiled:
        return _compiled["nc"]
    import concourse.mybir as mybir
    import concourse.bacc as bacc
    from concourse import tile

    FP = mybir.dt.float32
    BF = mybir.dt.bfloat16
    AF = mybir.ActivationFunctionType
    OP = mybir.AluOpType

    nc = bacc.Bacc("TRN2", target_bir_lowering=False, debug=False)

    xb_d = nc.dram_tensor("xb", (N, D), BF, kind="ExternalInput").ap()
    xbt_d = nc.dram_tensor("xbt", (D, N), BF, kind="ExternalInput").ap()
    vet_d = nc.dram_tensor("vet", (T, T), FP, kind="ExternalInput").ap()
    be_d = nc.dram_tensor("be", (T, T), FP, kind="ExternalInput").ap()
    rep_d = nc.dram_tensor("rep", (T, DCH * 128), FP, kind="ExternalInput").ap()
    p32_d = nc.dram_tensor("p32", (128, 128), BF, kind="ExternalInput").ap()
    vst_d = nc.dram_tensor("vst", (N, N), BF, kind="ExternalInput").ap()
    bs_d = nc.dram_tensor("bs", (N, N), FP, kind="ExternalInput").ap()
    lt_d = nc.dram_tensor("lt", (N, N), BF, kind="ExternalInput").ap()
    w4_d = nc.dram_tensor("w4", (4, 128, 128), BF, kind="ExternalInput").ap()
    biast_d = nc.dram_tensor("biast", (128, 1), FP, kind="ExternalInput").ap()
    identb_d = nc.dram_tensor("identb", (128, 128), BF, kind="ExternalInput").ap()
    out_d = nc.dram_tensor("out", (GT, N), FP, kind="ExternalOutput").ap()

    with tile.TileContext(nc) as tc:
        with (
            tc.tile_pool(name="persist", bufs=1) as pp,
            tc.tile_pool(name="stream", bufs=1) as sp,
            tc.tile_pool(name="psum", bufs=2, space="PSUM") as ps,
            tc.tile_pool(name="psum1", bufs=1, space="PSUM") as ps1,
            tc.tile_pool(name="psumtr", bufs=2, space="PSUM") as pst,
        ):
            def copy3(idx, dst, src):
                if idx % 3 == 2:
                    nc.scalar.copy(dst, src)
                else:
                    nc.vector.tensor_copy(dst, src)

            def vg(idx):
                return nc.vector if idx % 2 == 0 else nc.gpsimd

            # ---- constants ----
            vet = pp.tile([T, T], FP, tag="vet")
            nc.sync.dma_start(vet[:], vet_d[:])
            be = pp.tile([T, T], FP, tag="be")
            nc.sync.dma_start(be[:], be_d[:])
            rep = pp.tile([T, DCH * 128], FP, tag="rep")
            nc.sync.dma_start(rep[:], rep_d[:])
            p32 = pp.tile([128, 128], BF, tag="p32")
            nc.sync.dma_start(p32[:], p32_d[:])
            w4 = [pp.tile([128, 128], BF, name=f"w4{k}", tag=f"w4{k}") for k in range(4)]
            for k in range(4):
                nc.sync.dma_start(w4[k][:], w4_d[k])
            biast = pp.tile([128, 1], FP, tag="biast")
            nc.sync.dma_start(biast[:], biast_d[:])
            identb = pp.tile([128, 128], BF, tag="identb")
            nc.sync.dma_start(identb[:], identb_d[:])

            tr_idx = [0]
            tr_tile = [None]

            def pe_transpose(dst, src):
                j = tr_idx[0] % 4
                if j == 0:
                    tr_tile[0] = pst.tile([128, 512], BF,
                                          name=f"trt{tr_idx[0]}", tag="ps_tr")
                pt = tr_tile[0][:, j * 128:(j + 1) * 128]
                nc.tensor.transpose(pt, src, identb[:])
                copy3(tr_idx[0], dst, pt)
                tr_idx[0] += 1

            # ---- stage 0: inputs ----
            XN = []
            for i in range(NCH):
                t = pp.tile([128, D], BF, name=f"xnA{i}", tag=f"A{i}")
                nc.sync.dma_start(t[:], xb_d[i * 128:(i + 1) * 128, :])
                XN.append(t)
            XT = []
            for p in range(DCH):
                t = pp.tile([128, N], BF, name=f"xt{p}", tag=f"xt{p}")
                nc.sync.dma_start(t[:], xbt_d[p * 128:(p + 1) * 128, :])
                XT.append(t)
            VST = [pp.tile([128, N], BF, name=f"vstE{m}", tag=f"E{m}") for m in range(NCH)]
            for m in range(NCH):
                nc.sync.dma_start(VST[m][:], vst_d[m * 128:(m + 1) * 128, :])
            LT = [pp.tile([128, N], BF, name=f"lt{m}", tag=f"lt{m}") for m in range(NCH)]
            for m in range(NCH):
                nc.sync.dma_start(LT[m][:], lt_d[m * 128:(m + 1) * 128, :])

            # ---- stage 1: score_t (bf16, f-major slices of XN) ----
            ps_t = ps1.tile([T, T], FP, tag="ps_t")
            n_mm = NCH * F
            idx = 0
            for i in range(NCH):
                for f in range(F):
                    sl = XN[i][:, f * T:(f + 1) * T]
                    nc.tensor.matmul(ps_t[:], sl, sl,
                                     start=(idx == 0), stop=(idx == n_mm - 1))
                    idx += 1
            sig_t = pp.tile([T, T], FP, tag="sig_t")
            nc.scalar.activation(sig_t[:], ps_t[:], AF.Sigmoid)

            # ---- stage 2: E_att (no max-sub; logits bounded) ----
            ps_e = ps1.tile([T, T], FP, tag="ps_t")
            nc.tensor.matmul(ps_e[:], vet[:], sig_t[:], start=True, stop=True)
            epre = pp.tile([T, T], FP, tag="epre")
            nc.vector.tensor_tensor(epre[:], ps_e[:], be[:], op=OP.add)
            eexp = pp.tile([T, T], FP, tag="eexp")
            esum = pp.tile([T, 1], FP, tag="esum")
            nc.scalar.activation(eexp[:], epre[:], AF.Exp, accum_out=esum[:])
            einv = pp.tile([T, 1], FP, tag="einv")
            nc.vector.reciprocal(einv[:], esum[:])
            eatt = pp.tile([T, T], FP, tag="eatt")
            nc.vector.tensor_scalar_mul(eatt[:], eexp[:], einv[:])

            # E_EXP_p (128, 24) = REP_p.T @ eatt ; E2[p] (128, 768) Kron chunks
            EX = [pp.tile([128, T], FP, name=f"ex{p}", tag=f"ex{p}") for p in range(DCH)]
            for p in range(DCH):
                pe = ps1.tile([128, T], FP, tag="ps_t")
                nc.tensor.matmul(pe[:], rep[:, p * 128:(p + 1) * 128], eatt[:],
                                 start=True, stop=True)
                copy3(p, EX[p][:], pe[:])
            E2 = [pp.tile([128, D], BF, name=f"e2B{p}", tag=f"B{p}") for p in range(DCH)]
            p32v = p32[:].rearrange("r (b j) -> r b j", b=4, j=32)
            for q in range(DCH):
                for p in range(DCH):
                    dst = E2[p][:, q * 128:(q + 1) * 128].rearrange(
                        "r (b j) -> r b j", b=4, j=32)
                    src = EX[p][:, 4 * q:4 * q + 4].broadcast_to((128, 4, 32))
                    vg(p * DCH + q).tensor_tensor(dst, p32v, src, op=OP.mult)

            # ---- stage 3: TT' = sum_p E2[p][:,q].T @ XT[p] ----
            TT = [pp.tile([128, N], BF, name=f"ttC{q}", tag=f"C{q}") for q in range(DCH)]
            for q in range(DCH):
                for h in range(2):
                    pt = ps.tile([128, 512], FP, tag="ps_big")
                    for p in range(DCH):
                        nc.tensor.matmul(pt[:], E2[p][:, q * 128:(q + 1) * 128],
                                         XT[p][:, h * 512:(h + 1) * 512],
                                         start=(p == 0), stop=(p == DCH - 1))
                    copy3(q * 2 + h, TT[q][:, h * 512:(h + 1) * 512], pt[:])

            # ---- stage 4: TN (natural x_TA) <- dma transposes of TT ----
            TN = []
            for i in range(NCH):
                tag = f"B{i}" if i < DCH else f"tn{i}"
                TN.append(pp.tile([128, D], BF, name=f"tn{i}", tag=tag))
            for p in range(DCH):
                for i in range(NCH):
                    pe_transpose(TN[i][:, p * 128:(p + 1) * 128],
                                 TT[p][:, i * 128:(i + 1) * 128])

            # ---- stage 5: SG = sigmoid(score_s) ----
            SG = [pp.tile([128, N], BF, name=f"sgD{i}", tag=f"D{i}") for i in range(NCH)]
            for i in range(NCH):
                for h in range(2):
                    pt = ps.tile([128, 512], FP, tag="ps_big")
                    for p in range(DCH):
                        nc.tensor.matmul(pt[:], TT[p][:, i * 128:(i + 1) * 128],
                                         TT[p][:, h * 512:(h + 1) * 512],
                                         start=(p == 0), stop=(p == DCH - 1))
                    nc.scalar.activation(SG[i][:, h * 512:(h + 1) * 512],
                                         pt[:], AF.Sigmoid)

            # ---- stage 6: eexp_i = exp(Vs@SG + bs); SAT via dma transpose ----
            SAT = [pp.tile([128, N], BF, name=f"satA{m}", tag=f"A{m}") for m in range(NCH)]
            SINV = [pp.tile([128, 1], FP, name=f"sinv{i}", tag=f"sinv{i}")
                    for i in range(NCH)]
            for i in range(NCH):
                spre = sp.tile([128, N], FP, tag="spre", bufs=2)
                bsb = sp.tile([128, N], FP, tag="bsb", bufs=2)
                nc.sync.dma_start(bsb[:], bs_d[i * 128:(i + 1) * 128, :])
                for h in range(2):
                    pt = ps.tile([128, 512], FP, tag="ps_big")
                    for m in range(NCH):
                        nc.tensor.matmul(pt[:], VST[m][:, i * 128:(i + 1) * 128],
                                         SG[m][:, h * 512:(h + 1) * 512],
                                         start=(m == 0), stop=(m == NCH - 1))
                    nc.vector.tensor_tensor(spre[:, h * 512:(h + 1) * 512], pt[:],
                                            bsb[:, h * 512:(h + 1) * 512], op=OP.add)
                sexp = sp.tile([128, N], BF, tag="sexp", bufs=2)
                ssum = sp.tile([128, 1], FP, tag="ssum", bufs=2)
                nc.scalar.activation(sexp[:], spre[:], AF.Exp, accum_out=ssum[:])
                nc.vector.reciprocal(SINV[i][:], ssum[:])
                for m in range(NCH):
                    pe_transpose(SAT[m][:, i * 128:(i + 1) * 128],
                                 sexp[:, m * 128:(m + 1) * 128])

            # ---- stage 7: SN = sinv * (SAT.T @ TN) ----
            SN = [pp.tile([128, D], BF, name=f"snF{i}", tag=f"F{i}") for i in range(NCH)]
            for i in range(NCH):
                pta = ps.tile([128, 512], FP, name=f"pta{i}", tag="ps_big")
                ptb = ps.tile([128, 256], FP, name=f"ptb{i}", tag="ps_med")
                for pt, c0, cw in ((pta, 0, 512), (ptb, 512, 256)):
                    for m in range(NCH):
                        nc.tensor.matmul(pt[:, :cw],
                                         SAT[m][:, i * 128:(i + 1) * 128],
                                         TN[m][:, c0:c0 + cw],
                                         start=(m == 0), stop=(m == NCH - 1))
                nc.vector.tensor_scalar_mul(SN[i][:, 0:512], pta[:], SINV[i][:])
                nc.vector.tensor_scalar_mul(SN[i][:, 512:768], ptb[:], SINV[i][:])

            # ---- stage 8: SNT <- dma transposes of SN (reuse C slots) ----
            SNT = [pp.tile([128, N], BF, name=f"sntC{q}", tag=f"C{q}") for q in range(DCH)]
            for i in range(NCH):
                for q in range(DCH):
                    pe_transpose(SNT[q][:, i * 128:(i + 1) * 128],
                                 SN[i][:, q * 128:(q + 1) * 128])

            # ---- stage 9: P1 = L @ SN (natural) ----
            P1 = [pp.tile([128, D], BF, name=f"p1G{i}", tag=f"G{i}") for i in range(NCH)]
            for i in range(NCH):
                pta = ps.tile([128, 512], FP, name=f"pta{i}", tag="ps_big")
                ptb = ps.tile([128, 256], FP, name=f"ptb{i}", tag="ps_med")
                for pt, c0, cw in ((pta, 0, 512), (ptb, 512, 256)):
                    for m in range(NCH):
                        nc.tensor.matmul(pt[:, :cw],
                                         LT[m][:, i * 128:(i + 1) * 128],
                                         SN[m][:, c0:c0 + cw],
                                         start=(m == 0), stop=(m == NCH - 1))
                copy3(i, P1[i][:, 0:512], pta[:])
                copy3(i + 1, P1[i][:, 512:768], ptb[:])

            # ---- stages 10-12 interleaved per d-chunk q ----
            P1T = [pp.tile([128, N], BF, name=f"p1tD{q}", tag=f"D{q}") for q in range(DCH)]
            P2T = [pp.tile([128, N], BF, name=f"p2tE{q}", tag=f"E{q}") for q in range(DCH)]
            for q in range(DCH):
                for i in range(NCH):
                    pe_transpose(P1T[q][:, i * 128:(i + 1) * 128],
                                 P1[i][:, q * 128:(q + 1) * 128])
                for h in range(2):
                    pt = ps.tile([128, 512], FP, name=f"p2ps{q}{h}", tag="ps_big")
                    for m in range(NCH):
                        nc.tensor.matmul(pt[:], P1[m][:, q * 128:(q + 1) * 128],
                                         LT[m][:, h * 512:(h + 1) * 512],
                                         start=(m == 0), stop=(m == NCH - 1))
                    nc.vector.scalar_tensor_tensor(
                        P2T[q][:, h * 512:(h + 1) * 512], pt[:], 2.0,
                        SNT[q][:, h * 512:(h + 1) * 512],
                        op0=OP.mult, op1=OP.subtract)
                for b in range(2):
                    c = 2 * q + b
                    r0 = 64 * b
                    ob = sp.tile([128, N], FP, tag="outbuf", bufs=3)
                    srcs = (SNT[q], P1T[q], P2T[q], XT[q])
                    for h in range(2):
                        pt = ps.tile([128, 512], FP, name=f"ops{c}{h}", tag="ps_big")
                        for k in range(4):
                            nc.tensor.matmul(pt[:], w4[k][r0:r0 + 64, :],
                                             srcs[k][r0:r0 + 64, h * 512:(h + 1) * 512],
                                             start=(k == 0), stop=(k == 3))
                        dst = ob[:, h * 512:(h + 1) * 512]
                        if (c * 2 + h) % 2 == 0:
                            nc.scalar.activation(dst, pt[:], AF.Relu, bias=biast[:])
                        else:
                            nc.vector.tensor_scalar(dst, pt[:], biast[:], 0.0,
                                                    op0=OP.add, op1=OP.max)
                    nc.sync.dma_start(out_d[c * 128:(c + 1) * 128, :], ob[:])

    nc.compile()
    _compiled["nc"] = nc
    return nc


def _host_prep(x, edge_index, edge_weight, Ve, be, Vs, bs, cheb_W, cheb_b, res_W, res_b):
    import ml_dtypes
    BF = ml_dtypes.bfloat16
    row = np.asarray(edge_index[0]).astype(np.int64)
    col = np.asarray(edge_index[1]).astype(np.int64)
    w = np.asarray(edge_weight, np.float64).copy()
    w[row == col] = 0.0
    deg = np.zeros(N, np.float64)
    np.add.at(deg, row, w)
    dis = np.where(deg > 0, 1.0 / np.sqrt(np.where(deg > 0, deg, 1.0)), 0.0)
    norm = -dis[row] * w * dis[col]
    L = np.zeros((N, N), np.float64)
    np.add.at(L, (col, row), norm)

    cheb_W = np.asarray(cheb_W, np.float32)   # (K, F, G)
    res_W = np.asarray(res_W, np.float32)     # (G, F)
    Wk = [cheb_W[0], cheb_W[1], cheb_W[2], res_W.T]
    w4 = np.zeros((4, 128, 128), np.float32)
    for k in range(4):
        for c4 in range(4):
            c2 = c4 % 2
            w4[k, c4 * 32:(c4 + 1) * 32, c2 * 64:(c2 + 1) * 64] = Wk[k]
    b1 = (np.asarray(cheb_b, np.float32) + np.asarray(res_b, np.float32))
    biast = np.tile(b1, 2).reshape(128, 1).astype(np.float32)

    rep = np.zeros((T, DCH * 128), np.float32)
    for p in range(DCH):
        for a in range(4):
            u = 4 * p + a
            rep[u, p * 128 + 32 * a: p * 128 + 32 * a + 32] = 1.0
    p32 = np.zeros((128, 128), np.float32)
    for a in range(4):
        for b_ in range(4):
            p32[a * 32:(a + 1) * 32, b_ * 32:(b_ + 1) * 32] = np.eye(32)

    return {
        "vet": np.ascontiguousarray(np.asarray(Ve, np.float32).T),
        "be": np.ascontiguousarray(np.asarray(be, np.float32)[0]),
        "rep": rep,
        "p32": p32.astype(BF),
        "vst": np.ascontiguousarray(np.asarray(Vs, np.float32).T).astype(BF),
        "bs": np.ascontiguousarray(np.asarray(bs, np.float32)[0]),
        "lt": np.ascontiguousarray(L.T.astype(np.float32)).astype(BF),
        "w4": w4.astype(BF),
        "biast": biast,
        "identb": np.eye(128, dtype=np.float32).astype(BF),
    }


TRACE = False
LAST = {}


def kernel(x, edge_index, edge_weight, Ve, be, Vs, bs, cheb_W, cheb_b, res_W, res_b):
    import ml_dtypes
    from concourse.bass_utils import run_bass_kernel_spmd
    BF = ml_dtypes.bfloat16

    x = np.asarray(x, np.float32)
    shared = _host_prep(x, edge_index, edge_weight, Ve, be, Vs, bs,
                        cheb_W, cheb_b, res_W, res_b)
    nc = _build()
    in_maps = []
    for b in range(B):
        m = dict(shared)
        m["xb"] = np.ascontiguousarray(x[b].reshape(N, D)).astype(BF)
        # xbt: row d' = t*32+f  ->  x[b][n, f, t];  (D, N)
        m["xbt"] = np.ascontiguousarray(
            x[b].transpose(2, 1, 0).reshape(D, N)).astype(BF)
        in_maps.append(m)
    res = run_bass_kernel_spmd(nc, in_maps, list(range(B)), trace=TRACE)
    LAST["res"] = res
    # out (1536, 1024): row = c*128 + a*64 + g, t = 2c+a
    out = np.stack(
        [r["out"].reshape(12, 2, G, N).transpose(3, 2, 0, 1).reshape(N, G, T)
         for r in res.results], axis=0)
    return out


# revision 14
# speedup vs baseline: 1.0105x; 1.0105x over previous
"""STBlock (temporal attn -> spatial attn -> ChebConv + residual, relu) on 8 trn2 cores.

Sharding: data-parallel over batch B=8, one batch element per core.

v2 design: t-major feature layout d' = t*32 + f everywhere, zero PE transposes.
  - host uploads x twice in bf16: natural f-major (score_t) and pre-transposed
    t-major (768, 1024) for XT.
  - temporal attention applied in transposed space: TT' = (E_att^T (x) I32) @ XT
    via an on-device Kronecker-expanded E'' (built with 6 tiny matmuls + 36
    broadcast multiplies).
  - all on-device transposes (TN, SAT, SNT, P1T) are DMA xbar block transposes
    (bf16 128x128) on otherwise-idle DMA engines.
  - softmax skips max-subtraction (logits bounded by |Vs| row sums) and defers
    normalization into the SN psum evacuation scale (per-partition 1/sum).
  - final Cheb+residual projection consumes 64-row (2t x 32f) strips of the
    t-major transposed tensors against block-diag weights: 96 K=64 matmuls,
    output produced transposed (1536, 1024) and un-transposed on host.

Per-core dataflow:
  XN[8]  (128n, 768=(t,f)) bf16   <- dma xb16  (f-major natural, score_t only)
  XT[6]  (128d', 1024n) bf16      <- dma xb16T (host-transposed t-major)
  score_t (24,24) = 256 bf16 MMs; E_att = softmax(Ve sigmoid(score_t) + be)
  E2[6]  (128, 768) bf16 = E'' chunks (Kron expand of eatt via REP-matmuls + P32)
  TT[6]  (128d', 1024n) bf16 = sum_p E2[p,q].T @ XT[p]   (72 MMs)
  TN[8]  (128n, 768d') bf16 <- 48 dma transposes of TT
  SG[8]  (128, 1024) bf16 = sigmoid(TT.T TT)             (96 MMs)
  eexp_i (128, 1024) bf16 = exp(Vs@SG + bs), row sums -> sinv[i] (128 MMs)
  SAT[8] (128m, 1024n) bf16 <- 64 dma transposes of eexp
  SN[8]  (128n, 768) bf16 = sinv * SAT.T @ TN            (128 MMs)
  SNT[6] <- 48 dma transposes; P1[8] = LT.T @ SN (128 MMs); P1T[6] <- 48 dma T
  P2T[6] = 2*(P1.T@LT per chunk) - SNT                   (96 MMs)
  OUTT chunks c=0..11 (128=(2t,64g), 1024n): 4 accumulated K=64 MMs per half
    vs block-diag W4 (SNT,P1T,P2T,XT) -> relu(+bias) -> dma (1536,1024) fp32
Host un-transposes the output.
"""
import numpy as np

B, N, F, T, G = 8, 1024, 32, 24, 64
D = F * T            # 768
NCH = N // 128       # 8 n-chunks
DCH = D // 128       # 6 d-chunks
GT = G * T           # 1536

_compiled = {}


def _build():
    if "nc" in _compiled:
        return _compiled["nc"]
    import concourse.mybir as mybir
    import concourse.bacc as bacc
    from concourse import tile

    FP = mybir.dt.float32
    BF = mybir.dt.bfloat16
    AF = mybir.ActivationFunctionType
    OP = mybir.AluOpType

    nc = bacc.Bacc("TRN2", target_bir_lowering=False, debug=False)

    xb_d = nc.dram_tensor("xb", (N, D), BF, kind="ExternalInput").ap()
    xbt_d = nc.dram_tensor("xbt", (D, N), BF, kind="ExternalInput").ap()
    vet_d = nc.dram_tensor("vet", (T, T), FP, kind="ExternalInput").ap()
    be_d = nc.dram_tensor("be", (T, T), FP, kind="ExternalInput").ap()
    rep_d = nc.dram_tensor("rep", (T, DCH * 128), FP, kind="ExternalInput").ap()
    p32_d = nc.dram_tensor("p32", (128, 128), BF, kind="ExternalInput").ap()
    vst_d = nc.dram_tensor("vst", (N, N), BF, kind="ExternalInput").ap()
    bst_d = nc.dram_tensor("bst", (N, N), FP, kind="ExternalInput").ap()
    lt_d = nc.dram_tensor("lt", (N, N), BF, kind="ExternalInput").ap()
    w4_d = nc.dram_tensor("w4", (4, 128, 128), BF, kind="ExternalInput").ap()
    biast_d = nc.dram_tensor("biast", (128, 1), FP, kind="ExternalInput").ap()
    identb_d = nc.dram_tensor("identb", (128, 128), BF, kind="ExternalInput").ap()
    sel4_d = nc.dram_tensor("sel4", (128, T), FP, kind="ExternalInput").ap()
    out_d = nc.dram_tensor("out", (GT, N), FP, kind="ExternalOutput").ap()

    with tile.TileContext(nc) as tc:
        with (
            tc.tile_pool(name="persist", bufs=1) as pp,
            tc.tile_pool(name="stream", bufs=1) as sp,
            tc.tile_pool(name="psum", bufs=2, space="PSUM") as ps,
            tc.tile_pool(name="psum1", bufs=1, space="PSUM") as ps1,
            tc.tile_pool(name="psumtr", bufs=2, space="PSUM") as pst,
        ):
            def copy3(idx, dst, src):
                if idx % 3 == 2:
                    nc.scalar.copy(dst, src)
                else:
                    nc.vector.tensor_copy(dst, src)

            def vg(idx):
                return nc.vector if idx % 2 == 0 else nc.gpsimd

            # ---- constants ----
            vet = pp.tile([T, T], FP, tag="vet")
            nc.sync.dma_start(vet[:], vet_d[:])
            be = pp.tile([T, T], FP, tag="be")
            nc.sync.dma_start(be[:], be_d[:])
            rep = pp.tile([T, DCH * 128], FP, tag="rep")
            nc.sync.dma_start(rep[:], rep_d[:])
            p32 = pp.tile([128, 128], BF, tag="p32")
            nc.sync.dma_start(p32[:], p32_d[:])
            w4 = [pp.tile([128, 128], BF, name=f"w4{k}", tag=f"w4{k}") for k in range(4)]
            for k in range(4):
                nc.sync.dma_start(w4[k][:], w4_d[k])
            biast = pp.tile([128, 1], FP, tag="biast")
            nc.sync.dma_start(biast[:], biast_d[:])
            identb = pp.tile([128, 128], BF, tag="identb")
            nc.sync.dma_start(identb[:], identb_d[:])
            sel4 = pp.tile([128, T], FP, tag="sel4")
            nc.sync.dma_start(sel4[:], sel4_d[:])

            tr_idx = [0]
            tr_tile = [None]

            def pe_transpose(dst, src):
                j = tr_idx[0] % 4
                if j == 0:
                    tr_tile[0] = pst.tile([128, 512], BF,
                                          name=f"trt{tr_idx[0]}", tag="ps_tr")
                pt = tr_tile[0][:, j * 128:(j + 1) * 128]
                nc.tensor.transpose(pt, src, identb[:])
                copy3(tr_idx[0], dst, pt)
                tr_idx[0] += 1

            # ---- stage 0: inputs ----
            XN = []
            for i in range(NCH):
                t = pp.tile([128, D], BF, name=f"xnA{i}", tag=f"A{i}")
                nc.sync.dma_start(t[:], xb_d[i * 128:(i + 1) * 128, :])
                XN.append(t)
            XT = []
            for p in range(DCH):
                t = pp.tile([128, N], BF, name=f"xt{p}", tag=f"xt{p}")
                nc.sync.dma_start(t[:], xbt_d[p * 128:(p + 1) * 128, :])
                XT.append(t)
            VST = [pp.tile([128, N], BF, name=f"vstE{m}", tag=f"E{m}") for m in range(NCH)]
            for m in range(NCH):
                nc.sync.dma_start(VST[m][:], vst_d[m * 128:(m + 1) * 128, :])
            LT = [pp.tile([128, N], BF, name=f"lt{m}", tag=f"lt{m}") for m in range(NCH)]
            for m in range(NCH):
                nc.sync.dma_start(LT[m][:], lt_d[m * 128:(m + 1) * 128, :])

            # ---- stage 1: score_t, 4x col-tiled into one (128,24) psum ----
            ps4 = ps1.tile([128, 512], FP, tag="ps_t")
            for i in range(NCH):
                for f in range(F):
                    j = f % 4
                    sl = XN[i][:, f * T:(f + 1) * T]
                    nc.tensor.matmul(ps4[32 * j:32 * j + T, 0:T], sl, sl,
                                     tile_position=(0, 32 * j),
                                     start=(i == 0 and f < 4),
                                     stop=(i == NCH - 1 and f >= F - 4),
                                     skip_group_check=True)
            s4 = pp.tile([128, T], FP, tag="s4")
            for j in range(4):
                copy3(j, s4[32 * j:32 * j + T, :], ps4[32 * j:32 * j + T, 0:T])
            ps_t = ps1.tile([T, T], FP, tag="ps_t2")
            for j in range(4):
                nc.tensor.matmul(ps_t[:], sel4[32 * j:32 * j + T, :],
                                 s4[32 * j:32 * j + T, :],
                                 tile_position=(32 * j, 0),
                                 start=(j == 0), stop=(j == 3))
            sig_t = pp.tile([T, T], FP, tag="sig_t")
            nc.scalar.activation(sig_t[:], ps_t[:], AF.Sigmoid)

            # ---- stage 2: E_att (no max-sub; logits bounded) ----
            ps_e = ps1.tile([T, T], FP, tag="ps_t2")
            nc.tensor.matmul(ps_e[:], vet[:], sig_t[:], start=True, stop=True)
            epre = pp.tile([T, T], FP, tag="epre")
            nc.vector.tensor_tensor(epre[:], ps_e[:], be[:], op=OP.add)
            eexp = pp.tile([T, T], FP, tag="eexp")
            esum = pp.tile([T, 1], FP, tag="esum")
            nc.scalar.activation(eexp[:], epre[:], AF.Exp, accum_out=esum[:])
            einv = pp.tile([T, 1], FP, tag="einv")
            nc.vector.reciprocal(einv[:], esum[:])
            eatt = pp.tile([T, T], FP, tag="eatt")
            nc.vector.tensor_scalar_mul(eatt[:], eexp[:], einv[:])

            # E_EXP_p (128, 24) = REP_p.T @ eatt ; E2[p] (128, 768) Kron chunks
            EX = [pp.tile([128, T], FP, name=f"ex{p}", tag=f"ex{p}") for p in range(DCH)]
            for p in range(DCH):
                pe = ps1.tile([128, T], FP, tag="ps_t")
                nc.tensor.matmul(pe[:], rep[:, p * 128:(p + 1) * 128], eatt[:],
                                 start=True, stop=True)
                copy3(p, EX[p][:], pe[:])
            E2 = [pp.tile([128, D], BF, name=f"e2B{p}", tag=f"B{p}") for p in range(DCH)]
            p32v = p32[:].rearrange("r (b j) -> r b j", b=4, j=32)
            for q in range(DCH):
                for p in range(DCH):
                    dst = E2[p][:, q * 128:(q + 1) * 128].rearrange(
                        "r (b j) -> r b j", b=4, j=32)
                    src = EX[p][:, 4 * q:4 * q + 4].broadcast_to((128, 4, 32))
                    vg(p * DCH + q).tensor_tensor(dst, p32v, src, op=OP.mult)

            # ---- stage 3: TT' = sum_p E2[p][:,q].T @ XT[p] ----
            TT = [pp.tile([128, N], BF, name=f"ttC{q}", tag=f"C{q}") for q in range(DCH)]
            for q in range(DCH):
                for h in range(2):
                    pt = ps.tile([128, 512], FP, tag="ps_big")
                    for p in range(DCH):
                        nc.tensor.matmul(pt[:], E2[p][:, q * 128:(q + 1) * 128],
                                         XT[p][:, h * 512:(h + 1) * 512],
                                         start=(p == 0), stop=(p == DCH - 1))
                    copy3(q * 2 + h, TT[q][:, h * 512:(h + 1) * 512], pt[:])

            # ---- stage 4: TN (natural x_TA) <- dma transposes of TT ----
            TN = []
            for i in range(NCH):
                tag = f"B{i}" if i < DCH else f"tn{i}"
                TN.append(pp.tile([128, D + 8], BF, name=f"tn{i}", tag=tag))
            for i in range(NCH):
                nc.gpsimd.memset(TN[i][:, D:D + 1], 1.0)
            for p in range(DCH):
                for i in range(NCH):
                    pe_transpose(TN[i][:, p * 128:(p + 1) * 128],
                                 TT[p][:, i * 128:(i + 1) * 128])

            # ---- stage 5: SG = sigmoid(score_s) ----
            SG = [pp.tile([128, N], BF, name=f"sgD{i}", tag=f"D{i}") for i in range(NCH)]
            for i in range(NCH):
                for h in range(2):
                    pt = ps.tile([128, 512], FP, tag="ps_big")
                    for p in range(DCH):
                        nc.tensor.matmul(pt[:], TT[p][:, i * 128:(i + 1) * 128],
                                         TT[p][:, h * 512:(h + 1) * 512],
                                         start=(p == 0), stop=(p == DCH - 1))
                    nc.scalar.activation(SG[i][:, h * 512:(h + 1) * 512],
                                         pt[:], AF.Sigmoid)

            # ---- stage 6: transposed-first: SAT[j] = exp(SG.T@VsT + bsT) ----
            SAT = [pp.tile([128, N], BF, name=f"satA{j}", tag=f"A{j}") for j in range(NCH)]
            for j in range(NCH):
                spre = sp.tile([128, N], FP, tag="spre", bufs=2)
                bsb = sp.tile([128, N], FP, tag="bsb", bufs=2)
                nc.sync.dma_start(bsb[:], bst_d[j * 128:(j + 1) * 128, :])
                for h in range(2):
                    pt = ps.tile([128, 512], FP, tag="ps_big")
                    for r in range(NCH):
                        nc.tensor.matmul(pt[:], SG[r][:, j * 128:(j + 1) * 128],
                                         VST[r][:, h * 512:(h + 1) * 512],
                                         start=(r == 0), stop=(r == NCH - 1))
                    nc.vector.tensor_tensor(spre[:, h * 512:(h + 1) * 512], pt[:],
                                            bsb[:, h * 512:(h + 1) * 512], op=OP.add)
                nc.scalar.activation(SAT[j][:], spre[:], AF.Exp)

            # ---- stage 7: SN = sinv * (SAT.T @ TN) ----
            SN = [pp.tile([128, D], BF, name=f"snF{i}", tag=f"F{i}") for i in range(NCH)]
            SINV = [pp.tile([128, 1], FP, name=f"sinv{i}", tag=f"sinv{i}")
                    for i in range(NCH)]
            for i in range(NCH):
                pta = ps.tile([128, 512], FP, name=f"pta{i}", tag="ps_big")
                ptb = ps.tile([128, 257], FP, name=f"ptb{i}", tag="ps_med")
                for pt, c0, cw in ((ptb, 512, 257), (pta, 0, 512)):
                    for m in range(NCH):
                        nc.tensor.matmul(pt[:, :cw],
                                         SAT[m][:, i * 128:(i + 1) * 128],
                                         TN[m][:, c0:c0 + cw],
                                         start=(m == 0), stop=(m == NCH - 1))
                nc.vector.reciprocal(SINV[i][:], ptb[:, 256:257])
                nc.vector.tensor_scalar_mul(SN[i][:, 0:512], pta[:], SINV[i][:])
                nc.vector.tensor_scalar_mul(SN[i][:, 512:768], ptb[:, 0:256],
                                            SINV[i][:])

            # ---- stage 8: SNT <- dma transposes of SN (reuse C slots) ----
            SNT = [pp.tile([128, N], BF, name=f"sntC{q}", tag=f"C{q}") for q in range(DCH)]
            for i in range(NCH):
                for q in range(DCH):
                    pe_transpose(SNT[q][:, i * 128:(i + 1) * 128],
                                 SN[i][:, q * 128:(q + 1) * 128])

            # ---- stage 9: P1 = L @ SN (natural) ----
            P1 = [pp.tile([128, D], BF, name=f"p1G{i}", tag=f"G{i}") for i in range(NCH)]
            for i in range(NCH):
                pta = ps.tile([128, 512], FP, name=f"pta{i}", tag="ps_big")
                ptb = ps.tile([128, 256], FP, name=f"ptb{i}", tag="ps_med")
                for pt, c0, cw in ((pta, 0, 512), (ptb, 512, 256)):
                    for m in range(NCH):
                        nc.tensor.matmul(pt[:, :cw],
                                         LT[m][:, i * 128:(i + 1) * 128],
                                         SN[m][:, c0:c0 + cw],
                                         start=(m == 0), stop=(m == NCH - 1))
                copy3(i, P1[i][:, 0:512], pta[:])
                copy3(i + 1, P1[i][:, 512:768], ptb[:])

            # ---- stages 10-12 interleaved per d-chunk q ----
            P1T = [pp.tile([128, N], BF, name=f"p1tD{q}", tag=f"D{q}") for q in range(DCH)]
            P2T = [pp.tile([128, N], BF, name=f"p2tE{q}", tag=f"E{q}") for q in range(DCH)]
            for q in range(DCH):
                for i in range(NCH):
                    pe_transpose(P1T[q][:, i * 128:(i + 1) * 128],
                                 P1[i][:, q * 128:(q + 1) * 128])
                for h in range(2):
                    pt = ps.tile([128, 512], FP, name=f"p2ps{q}{h}", tag="ps_big")
                    for m in range(NCH):
                        nc.tensor.matmul(pt[:], P1[m][:, q * 128:(q + 1) * 128],
                                         LT[m][:, h * 512:(h + 1) * 512],
                                         start=(m == 0), stop=(m == NCH - 1))
                    nc.vector.scalar_tensor_tensor(
                        P2T[q][:, h * 512:(h + 1) * 512], pt[:], 2.0,
                        SNT[q][:, h * 512:(h + 1) * 512],
                        op0=OP.mult, op1=OP.subtract)
                for b in range(2):
                    c = 2 * q + b
                    r0 = 64 * b
                    ob = sp.tile([128, N], FP, tag="outbuf", bufs=3)
                    srcs = (SNT[q], P1T[q], P2T[q], XT[q])
                    for h in range(2):
                        pt = ps.tile([128, 512], FP, name=f"ops{c}{h}", tag="ps_big")
                        for k in range(4):
                            nc.tensor.matmul(pt[:], w4[k][r0:r0 + 64, :],
                                             srcs[k][r0:r0 + 64, h * 512:(h + 1) * 512],
                                             start=(k == 0), stop=(k == 3))
                        dst = ob[:, h * 512:(h + 1) * 512]
                        if (c * 2 + h) % 2 == 0:
                            nc.scalar.activation(dst, pt[:], AF.Relu, bias=biast[:])
                        else:
                            nc.vector.tensor_scalar(dst, pt[:], biast[:], 0.0,
                                                    op0=OP.add, op1=OP.max)
                    nc.sync.dma_start(out_d[c * 128:(c + 1) * 128, :], ob[:])

    nc.compile()
    _compiled["nc"] = nc
    return nc


def _host_prep(x, edge_index, edge_weight, Ve, be, Vs, bs, cheb_W, cheb_b, res_W, res_b):
    import ml_dtypes
    BF = ml_dtypes.bfloat16
    row = np.asarray(edge_index[0]).astype(np.int64)
    col = np.asarray(edge_index[1]).astype(np.int64)
    w = np.asarray(edge_weight, np.float64).copy()
    w[row == col] = 0.0
    deg = np.zeros(N, np.float64)
    np.add.at(deg, row, w)
    dis = np.where(deg > 0, 1.0 / np.sqrt(np.where(deg > 0, deg, 1.0)), 0.0)
    norm = -dis[row] * w * dis[col]
    L = np.zeros((N, N), np.float64)
    np.add.at(L, (col, row), norm)

    cheb_W = np.asarray(cheb_W, np.float32)   # (K, F, G)
    res_W = np.asarray(res_W, np.float32)     # (G, F)
    Wk = [cheb_W[0], cheb_W[1], cheb_W[2], res_W.T]
    w4 = np.zeros((4, 128, 128), np.float32)
    for k in range(4):
        for c4 in range(4):
            c2 = c4 % 2
            w4[k, c4 * 32:(c4 + 1) * 32, c2 * 64:(c2 + 1) * 64] = Wk[k]
    b1 = (np.asarray(cheb_b, np.float32) + np.asarray(res_b, np.float32))
    biast = np.tile(b1, 2).reshape(128, 1).astype(np.float32)

    rep = np.zeros((T, DCH * 128), np.float32)
    for p in range(DCH):
        for a in range(4):
            u = 4 * p + a
            rep[u, p * 128 + 32 * a: p * 128 + 32 * a + 32] = 1.0
    p32 = np.zeros((128, 128), np.float32)
    for a in range(4):
        for b_ in range(4):
            p32[a * 32:(a + 1) * 32, b_ * 32:(b_ + 1) * 32] = np.eye(32)

    return {
        "vet": np.ascontiguousarray(np.asarray(Ve, np.float32).T),
        "be": np.ascontiguousarray(np.asarray(be, np.float32)[0]),
        "rep": rep,
        "p32": p32.astype(BF),
        "vst": np.ascontiguousarray(np.asarray(Vs, np.float32).T).astype(BF),
        "bst": np.ascontiguousarray(np.asarray(bs, np.float32)[0].T),
        "lt": np.ascontiguousarray(L.T.astype(np.float32)).astype(BF),
        "w4": w4.astype(BF),
        "biast": biast,
        "identb": np.eye(128, dtype=np.float32).astype(BF),
        "sel4": np.vstack([np.vstack([np.eye(T, dtype=np.float32),
                                      np.zeros((8, T), np.float32)])
                           for _ in range(4)]),
    }


TRACE = False
LAST = {}


def kernel(x, edge_index, edge_weight, Ve, be, Vs, bs, cheb_W, cheb_b, res_W, res_b):
    import ml_dtypes
    from concourse.bass_utils import run_bass_kernel_spmd
    BF = ml_dtypes.bfloat16

    x = np.asarray(x, np.float32)
    shared = _host_prep(x, edge_index, edge_weight, Ve, be, Vs, bs,
                        cheb_W, cheb_b, res_W, res_b)
    nc = _build()
    in_maps = []
    for b in range(B):
        m = dict(shared)
        m["xb"] = np.ascontiguousarray(x[b].reshape(N, D)).astype(BF)
        # xbt: row d' = t*32+f  ->  x[b][n, f, t];  (D, N)
        m["xbt"] = np.ascontiguousarray(
            x[b].transpose(2, 1, 0).reshape(D, N)).astype(BF)
        in_maps.append(m)
    res = run_bass_kernel_spmd(nc, in_maps, list(range(B)), trace=TRACE)
    LAST["res"] = res
    # out (1536, 1024): row = c*128 + a*64 + g, t = 2c+a
    out = np.stack(
        [r["out"].reshape(12, 2, G, N).transpose(3, 2, 0, 1).reshape(N, G, T)
         for r in res.results], axis=0)
    return out


# revision 17
# speedup vs baseline: 1.1158x; 1.1042x over previous
"""STBlock (temporal attn -> spatial attn -> ChebConv + residual, relu) on 8 trn2 cores.

Sharding: data-parallel over batch B=8, one batch element per core.

v2 design: t-major feature layout d' = t*32 + f everywhere, zero PE transposes.
  - host uploads x twice in bf16: natural f-major (score_t) and pre-transposed
    t-major (768, 1024) for XT.
  - temporal attention applied in transposed space: TT' = (E_att^T (x) I32) @ XT
    via an on-device Kronecker-expanded E'' (built with 6 tiny matmuls + 36
    broadcast multiplies).
  - all on-device transposes (TN, SAT, SNT, P1T) are DMA xbar block transposes
    (bf16 128x128) on otherwise-idle DMA engines.
  - softmax skips max-subtraction (logits bounded by |Vs| row sums) and defers
    normalization into the SN psum evacuation scale (per-partition 1/sum).
  - final Cheb+residual projection consumes 64-row (2t x 32f) strips of the
    t-major transposed tensors against block-diag weights: 96 K=64 matmuls,
    output produced transposed (1536, 1024) and un-transposed on host.

Per-core dataflow:
  XN[8]  (128n, 768=(t,f)) bf16   <- dma xb16  (f-major natural, score_t only)
  XT[6]  (128d', 1024n) bf16      <- dma xb16T (host-transposed t-major)
  score_t (24,24) = 256 bf16 MMs; E_att = softmax(Ve sigmoid(score_t) + be)
  E2[6]  (128, 768) bf16 = E'' chunks (Kron expand of eatt via REP-matmuls + P32)
  TT[6]  (128d', 1024n) bf16 = sum_p E2[p,q].T @ XT[p]   (72 MMs)
  TN[8]  (128n, 768d') bf16 <- 48 dma transposes of TT
  SG[8]  (128, 1024) bf16 = sigmoid(TT.T TT)             (96 MMs)
  eexp_i (128, 1024) bf16 = exp(Vs@SG + bs), row sums -> sinv[i] (128 MMs)
  SAT[8] (128m, 1024n) bf16 <- 64 dma transposes of eexp
  SN[8]  (128n, 768) bf16 = sinv * SAT.T @ TN            (128 MMs)
  SNT[6] <- 48 dma transposes; P1[8] = LT.T @ SN (128 MMs); P1T[6] <- 48 dma T
  P2T[6] = 2*(P1.T@LT per chunk) - SNT                   (96 MMs)
  OUTT chunks c=0..11 (128=(2t,64g), 1024n): 4 accumulated K=64 MMs per half
    vs block-diag W4 (SNT,P1T,P2T,XT) -> relu(+bias) -> dma (1536,1024) fp32
Host un-transposes the output.
"""
import numpy as np

B, N, F, T, G = 8, 1024, 32, 24, 64
D = F * T            # 768
NCH = N // 128       # 8 n-chunks
DCH = D // 128       # 6 d-chunks
GT = G * T           # 1536

_compiled = {}


def _build():
    if "nc" in _compiled:
        return _compiled["nc"]
    import concourse.mybir as mybir
    import concourse.bacc as bacc
    from concourse import tile

    FP = mybir.dt.float32
    BF = mybir.dt.bfloat16
    F8 = mybir.dt.float8e4
    AF = mybir.ActivationFunctionType
    OP = mybir.AluOpType

    nc = bacc.Bacc("TRN2", target_bir_lowering=False, debug=False)

    xb_d = nc.dram_tensor("xb", (N, D), BF, kind="ExternalInput").ap()
    xbt_d = nc.dram_tensor("xbt", (D, N), BF, kind="ExternalInput").ap()
    vet_d = nc.dram_tensor("vet", (T, T), FP, kind="ExternalInput").ap()
    be_d = nc.dram_tensor("be", (T, T), FP, kind="ExternalInput").ap()
    rep_d = nc.dram_tensor("rep", (T, DCH * 128), FP, kind="ExternalInput").ap()
    p32_d = nc.dram_tensor("p32", (128, 128), BF, kind="ExternalInput").ap()
    vstp_d = nc.dram_tensor("vstp", (4, 128, 2 * N), F8, kind="ExternalInput").ap()
    bst_d = nc.dram_tensor("bst", (N, N), FP, kind="ExternalInput").ap()
    ltp_d = nc.dram_tensor("ltp", (4, 128, 2 * N), F8, kind="ExternalInput").ap()
    w4_d = nc.dram_tensor("w4", (4, 128, 128), BF, kind="ExternalInput").ap()
    biast_d = nc.dram_tensor("biast", (128, 1), FP, kind="ExternalInput").ap()
    identb_d = nc.dram_tensor("identb", (128, 128), BF, kind="ExternalInput").ap()
    sel4_d = nc.dram_tensor("sel4", (128, T), FP, kind="ExternalInput").ap()
    out_d = nc.dram_tensor("out", (GT, N), BF, kind="ExternalOutput").ap()

    with tile.TileContext(nc) as tc:
        with (
            tc.tile_pool(name="persist", bufs=1) as pp,
            tc.tile_pool(name="stream", bufs=1) as sp,
            tc.tile_pool(name="psum", bufs=2, space="PSUM") as ps,
            tc.tile_pool(name="psum1", bufs=1, space="PSUM") as ps1,
            tc.tile_pool(name="psumtr", bufs=2, space="PSUM") as pst,
        ):
            def copy3(idx, dst, src):
                if idx % 3 == 2:
                    nc.scalar.copy(dst, src)
                else:
                    nc.vector.tensor_copy(dst, src)

            def vg(idx):
                return nc.vector if idx % 2 == 0 else nc.gpsimd

            # ---- constants ----
            vet = pp.tile([T, T], FP, tag="vet")
            nc.sync.dma_start(vet[:], vet_d[:])
            be = pp.tile([T, T], FP, tag="be")
            nc.sync.dma_start(be[:], be_d[:])
            rep = pp.tile([T, DCH * 128], FP, tag="rep")
            nc.sync.dma_start(rep[:], rep_d[:])
            p32 = pp.tile([128, 128], BF, tag="p32")
            nc.sync.dma_start(p32[:], p32_d[:])
            w4 = [pp.tile([128, 128], BF, name=f"w4{k}", tag=f"w4{k}") for k in range(4)]
            for k in range(4):
                nc.sync.dma_start(w4[k][:], w4_d[k])
            biast = pp.tile([128, 1], FP, tag="biast")
            nc.sync.dma_start(biast[:], biast_d[:])
            identb = pp.tile([128, 128], BF, tag="identb")
            nc.sync.dma_start(identb[:], identb_d[:])
            sel4 = pp.tile([128, T], FP, tag="sel4")
            nc.sync.dma_start(sel4[:], sel4_d[:])

            tr_idx = [0]
            tr_tile = [None]

            def pe_transpose(dst, src):
                j = tr_idx[0] % 4
                if j == 0:
                    tr_tile[0] = pst.tile([128, 512], BF,
                                          name=f"trt{tr_idx[0]}", tag="ps_tr")
                pt = tr_tile[0][:, j * 128:(j + 1) * 128]
                nc.tensor.transpose(pt, src, identb[:])
                copy3(tr_idx[0], dst, pt)
                tr_idx[0] += 1

            # ---- stage 0: inputs ----
            XN = []
            for i in range(NCH):
                t = pp.tile([128, D], BF, name=f"xnA{i}", tag=f"A{i}")
                nc.sync.dma_start(t[:], xb_d[i * 128:(i + 1) * 128, :])
                XN.append(t)
            XT = []
            for p in range(DCH):
                t = pp.tile([128, N], BF, name=f"xt{p}", tag=f"xt{p}")
                nc.sync.dma_start(t[:], xbt_d[p * 128:(p + 1) * 128, :])
                XT.append(t)
            VSTP = [pp.tile([128, 2 * N], F8, name=f"vstp{r}", tag=f"vstp{r}")
                    for r in range(4)]
            for r in range(4):
                nc.sync.dma_start(VSTP[r][:], vstp_d[r])
            LTP = [pp.tile([128, 2 * N], F8, name=f"ltp{r}", tag=f"ltp{r}")
                   for r in range(4)]
            for r in range(4):
                nc.sync.dma_start(LTP[r][:], ltp_d[r])

            def pair3(tile_, c0, cw):
                return tile_[:].rearrange("p (two n) -> p two n", two=2)[:, :, c0:c0 + cw]
            DR = mybir.MatmulPerfMode.DoubleRow

            # ---- stage 1: score_t, 4x col-tiled into one (128,24) psum ----
            ps4 = ps1.tile([128, 512], FP, tag="ps_t")
            for i in range(NCH):
                for f in range(F):
                    j = f % 4
                    sl = XN[i][:, f * T:(f + 1) * T]
                    nc.tensor.matmul(ps4[32 * j:32 * j + T, 0:T], sl, sl,
                                     tile_position=(0, 32 * j),
                                     start=(i == 0 and f < 4),
                                     stop=(i == NCH - 1 and f >= F - 4),
                                     skip_group_check=True)
            s4 = pp.tile([128, T], FP, tag="s4")
            for j in range(4):
                copy3(j, s4[32 * j:32 * j + T, :], ps4[32 * j:32 * j + T, 0:T])
            ps_t = ps1.tile([T, T], FP, tag="ps_t2")
            for j in range(4):
                nc.tensor.matmul(ps_t[:], sel4[32 * j:32 * j + T, :],
                                 s4[32 * j:32 * j + T, :],
                                 tile_position=(32 * j, 0),
                                 start=(j == 0), stop=(j == 3))
            sig_t = pp.tile([T, T], FP, tag="sig_t")
            nc.scalar.activation(sig_t[:], ps_t[:], AF.Sigmoid)

            # ---- stage 2: E_att (no max-sub; logits bounded) ----
            ps_e = ps1.tile([T, T], FP, tag="ps_t2")
            nc.tensor.matmul(ps_e[:], vet[:], sig_t[:], start=True, stop=True)
            epre = pp.tile([T, T], FP, tag="epre")
            nc.vector.tensor_tensor(epre[:], ps_e[:], be[:], op=OP.add)
            eexp = pp.tile([T, T], FP, tag="eexp")
            esum = pp.tile([T, 1], FP, tag="esum")
            nc.scalar.activation(eexp[:], epre[:], AF.Exp, accum_out=esum[:])
            einv = pp.tile([T, 1], FP, tag="einv")
            nc.vector.reciprocal(einv[:], esum[:])
            eatt = pp.tile([T, T], FP, tag="eatt")
            nc.vector.tensor_scalar_mul(eatt[:], eexp[:], einv[:])

            # E_EXP_p (128, 24) = REP_p.T @ eatt ; E2[p] (128, 768) Kron chunks
            EX = [pp.tile([128, T], FP, name=f"ex{p}", tag=f"ex{p}") for p in range(DCH)]
            for p in range(DCH):
                pe = ps1.tile([128, T], FP, tag="ps_t")
                nc.tensor.matmul(pe[:], rep[:, p * 128:(p + 1) * 128], eatt[:],
                                 start=True, stop=True)
                copy3(p, EX[p][:], pe[:])
            E2 = [pp.tile([128, D], BF, name=f"e2B{p}", tag=f"B{p}") for p in range(DCH)]
            p32v = p32[:].rearrange("r (b j) -> r b j", b=4, j=32)
            for q in range(DCH):
                for p in range(DCH):
                    dst = E2[p][:, q * 128:(q + 1) * 128].rearrange(
                        "r (b j) -> r b j", b=4, j=32)
                    src = EX[p][:, 4 * q:4 * q + 4].broadcast_to((128, 4, 32))
                    vg(p * DCH + q).tensor_tensor(dst, p32v, src, op=OP.mult)

            # ---- stage 3: TT' = sum_p E2[p][:,q].T @ XT[p] ----
            TT = [pp.tile([128, N], BF, name=f"ttC{q}", tag=f"C{q}") for q in range(DCH)]
            TT8 = [pp.tile([128, 2 * N], F8, name=f"tt8{w}", tag=f"tt8{w}")
                   for w in range(3)]
            for q in range(DCH):
                for h in range(2):
                    pt = ps.tile([128, 512], FP, tag="ps_big")
                    for p in range(DCH):
                        nc.tensor.matmul(pt[:], E2[p][:, q * 128:(q + 1) * 128],
                                         XT[p][:, h * 512:(h + 1) * 512],
                                         start=(p == 0), stop=(p == DCH - 1))
                    copy3(q * 2 + h, TT[q][:, h * 512:(h + 1) * 512], pt[:])
                    copy3(q * 2 + h + 1,
                          TT8[q // 2][:, (q % 2) * N + h * 512:
                                      (q % 2) * N + (h + 1) * 512], pt[:])

            # ---- stage 4: TN (natural x_TA) <- dma transposes of TT ----
            TN = []
            for i in range(NCH):
                tag = f"B{i}" if i < DCH else f"tn{i}"
                TN.append(pp.tile([128, D + 8], BF, name=f"tn{i}", tag=tag))
            for i in range(NCH):
                nc.gpsimd.memset(TN[i][:, D:D + 1], 1.0)
            for p in range(DCH):
                for i in range(NCH):
                    pe_transpose(TN[i][:, p * 128:(p + 1) * 128],
                                 TT[p][:, i * 128:(i + 1) * 128])

            # ---- stage 5: SG = sigmoid(score_s), fp8 DoubleRow pairs ----
            SGP = [pp.tile([128, 2 * N], F8, name=f"sgp{r}", tag=f"sgp{r}")
                   for r in range(4)]
            for i in range(NCH):
                for h in range(2):
                    pt = ps.tile([128, 512], FP, tag="ps_big")
                    for w in range(3):
                        nc.tensor.matmul(pt[:],
                                         pair3(TT8[w], i * 128, 128),
                                         pair3(TT8[w], h * 512, 512),
                                         perf_mode=DR,
                                         start=(w == 0), stop=(w == 2))
                    nc.scalar.activation(
                        SGP[i // 2][:, (i % 2) * N + h * 512:
                                    (i % 2) * N + (h + 1) * 512],
                        pt[:], AF.Sigmoid)

            # ---- stage 6: transposed-first: SAT[j] = exp(SG.T@VsT + bsT) ----
            SAT = [pp.tile([128, N], BF, name=f"satA{j}", tag=f"A{j}") for j in range(NCH)]
            for j in range(NCH):
                spre = sp.tile([128, N], FP, tag="spre", bufs=2)
                bsb = sp.tile([128, N], FP, tag="bsb", bufs=2)
                nc.sync.dma_start(bsb[:], bst_d[j * 128:(j + 1) * 128, :])
                for h in range(2):
                    pt = ps.tile([128, 512], FP, tag="ps_big")
                    for r in range(4):
                        nc.tensor.matmul(pt[:],
                                         pair3(SGP[r], j * 128, 128),
                                         pair3(VSTP[r], h * 512, 512),
                                         perf_mode=DR,
                                         start=(r == 0), stop=(r == 3))
                    nc.vector.tensor_tensor(spre[:, h * 512:(h + 1) * 512], pt[:],
                                            bsb[:, h * 512:(h + 1) * 512], op=OP.add)
                nc.scalar.activation(SAT[j][:], spre[:], AF.Exp)

            # ---- stage 7: SN = sinv * (SAT.T @ TN) ----
            SN = [pp.tile([128, D], BF, name=f"snF{i}", tag=f"F{i}") for i in range(NCH)]
            SNP = [pp.tile([128, 2 * N], F8, name=f"snp{r}", tag=f"snp{r}")
                   for r in range(4)]
            SINV = [pp.tile([128, 1], FP, name=f"sinv{i}", tag=f"sinv{i}")
                    for i in range(NCH)]
            for i in range(NCH):
                pta = ps.tile([128, 512], FP, name=f"pta{i}", tag="ps_big")
                ptb = ps.tile([128, 257], FP, name=f"ptb{i}", tag="ps_med")
                for pt, c0, cw in ((ptb, 512, 257), (pta, 0, 512)):
                    for m in range(NCH):
                        nc.tensor.matmul(pt[:, :cw],
                                         SAT[m][:, i * 128:(i + 1) * 128],
                                         TN[m][:, c0:c0 + cw],
                                         start=(m == 0), stop=(m == NCH - 1))
                nc.vector.reciprocal(SINV[i][:], ptb[:, 256:257])
                nc.vector.tensor_scalar_mul(SN[i][:, 0:512], pta[:], SINV[i][:])
                nc.vector.tensor_scalar_mul(SN[i][:, 512:768], ptb[:, 0:256],
                                            SINV[i][:])
                o8 = (i % 2) * N
                nc.vector.tensor_scalar_mul(SNP[i // 2][:, o8:o8 + 512],
                                            pta[:], SINV[i][:])
                nc.vector.tensor_scalar_mul(SNP[i // 2][:, o8 + 512:o8 + 768],
                                            ptb[:, 0:256], SINV[i][:])

            # ---- stage 8: SNT <- dma transposes of SN (reuse C slots) ----
            SNT = [pp.tile([128, N], BF, name=f"sntC{q}", tag=f"C{q}") for q in range(DCH)]
            for i in range(NCH):
                for q in range(DCH):
                    pe_transpose(SNT[q][:, i * 128:(i + 1) * 128],
                                 SN[i][:, q * 128:(q + 1) * 128])

            # ---- stage 9: P1 = L @ SN (natural) ----
            P1 = [pp.tile([128, D], BF, name=f"p1G{i}", tag=f"G{i}") for i in range(NCH)]
            P1P = [pp.tile([128, 2 * N], F8, name=f"p1p{r}", tag=f"p1p{r}")
                   for r in range(4)]
            for i in range(NCH):
                pta = ps.tile([128, 512], FP, name=f"pta{i}", tag="ps_big")
                ptb = ps.tile([128, 256], FP, name=f"ptb{i}", tag="ps_med")
                for pt, c0, cw in ((pta, 0, 512), (ptb, 512, 256)):
                    for m in range(4):
                        nc.tensor.matmul(pt[:, :cw],
                                         pair3(LTP[m], i * 128, 128),
                                         pair3(SNP[m], c0, cw),
                                         perf_mode=DR,
                                         start=(m == 0), stop=(m == 3))
                copy3(i, P1[i][:, 0:512], pta[:])
                copy3(i + 1, P1[i][:, 512:768], ptb[:])
                o8 = (i % 2) * N
                copy3(i + 2, P1P[i // 2][:, o8:o8 + 512], pta[:])
                copy3(i + 3, P1P[i // 2][:, o8 + 512:o8 + 768], ptb[:])

            # ---- stages 10-12 interleaved per d-chunk q ----
            P1T = [pp.tile([128, N], BF, name=f"p1tD{q}", tag=f"D{q}") for q in range(DCH)]
            P2T = [pp.tile([128, N], BF, name=f"p2tE{q}", tag=f"E{q}") for q in range(DCH)]
            for q in range(DCH):
                for i in range(NCH):
                    pe_transpose(P1T[q][:, i * 128:(i + 1) * 128],
                                 P1[i][:, q * 128:(q + 1) * 128])
                for h in range(2):
                    pt = ps.tile([128, 512], FP, name=f"p2ps{q}{h}", tag="ps_big")
                    for m in range(4):
                        nc.tensor.matmul(pt[:],
                                         pair3(P1P[m], q * 128, 128),
                                         pair3(LTP[m], h * 512, 512),
                                         perf_mode=DR,
                                         start=(m == 0), stop=(m == 3))
                    nc.vector.scalar_tensor_tensor(
                        P2T[q][:, h * 512:(h + 1) * 512], pt[:], 2.0,
                        SNT[q][:, h * 512:(h + 1) * 512],
                        op0=OP.mult, op1=OP.subtract)
                for b in range(2):
                    c = 2 * q + b
                    r0 = 64 * b
                    ob = sp.tile([128, N], BF, tag="outbuf", bufs=3)
                    srcs = (SNT[q], P1T[q], P2T[q], XT[q])
                    for h in range(2):
                        pt = ps.tile([128, 512], FP, name=f"ops{c}{h}", tag="ps_big")
                        for k in range(4):
                            nc.tensor.matmul(pt[:], w4[k][r0:r0 + 64, :],
                                             srcs[k][r0:r0 + 64, h * 512:(h + 1) * 512],
                                             start=(k == 0), stop=(k == 3))
                        dst = ob[:, h * 512:(h + 1) * 512]
                        if (c * 2 + h) % 2 == 0:
                            nc.scalar.activation(dst, pt[:], AF.Relu, bias=biast[:])
                        else:
                            nc.vector.tensor_scalar(dst, pt[:], biast[:], 0.0,
                                                    op0=OP.add, op1=OP.max)
                    nc.sync.dma_start(out_d[c * 128:(c + 1) * 128, :], ob[:])

    nc.compile()
    _compiled["nc"] = nc
    return nc


def _host_prep(x, edge_index, edge_weight, Ve, be, Vs, bs, cheb_W, cheb_b, res_W, res_b):
    import ml_dtypes
    BF = ml_dtypes.bfloat16
    row = np.asarray(edge_index[0]).astype(np.int64)
    col = np.asarray(edge_index[1]).astype(np.int64)
    w = np.asarray(edge_weight, np.float64).copy()
    w[row == col] = 0.0
    deg = np.zeros(N, np.float64)
    np.add.at(deg, row, w)
    dis = np.where(deg > 0, 1.0 / np.sqrt(np.where(deg > 0, deg, 1.0)), 0.0)
    norm = -dis[row] * w * dis[col]
    L = np.zeros((N, N), np.float64)
    np.add.at(L, (col, row), norm)

    cheb_W = np.asarray(cheb_W, np.float32)   # (K, F, G)
    res_W = np.asarray(res_W, np.float32)     # (G, F)
    Wk = [cheb_W[0], cheb_W[1], cheb_W[2], res_W.T]
    w4 = np.zeros((4, 128, 128), np.float32)
    for k in range(4):
        for c4 in range(4):
            c2 = c4 % 2
            w4[k, c4 * 32:(c4 + 1) * 32, c2 * 64:(c2 + 1) * 64] = Wk[k]
    b1 = (np.asarray(cheb_b, np.float32) + np.asarray(res_b, np.float32))
    biast = np.tile(b1, 2).reshape(128, 1).astype(np.float32)

    rep = np.zeros((T, DCH * 128), np.float32)
    for p in range(DCH):
        for a in range(4):
            u = 4 * p + a
            rep[u, p * 128 + 32 * a: p * 128 + 32 * a + 32] = 1.0
    p32 = np.zeros((128, 128), np.float32)
    for a in range(4):
        for b_ in range(4):
            p32[a * 32:(a + 1) * 32, b_ * 32:(b_ + 1) * 32] = np.eye(32)

    import ml_dtypes as mld
    F8 = mld.float8_e4m3

    def pairs(mat):
        return np.ascontiguousarray(
            mat.reshape(4, 2, 128, N).transpose(0, 2, 1, 3).reshape(4, 128, 2 * N)
        ).astype(F8)

    return {
        "vstp": pairs(np.ascontiguousarray(np.asarray(Vs, np.float32).T)),
        "ltp": pairs(np.ascontiguousarray(L.T.astype(np.float32))),
        "vet": np.ascontiguousarray(np.asarray(Ve, np.float32).T),
        "be": np.ascontiguousarray(np.asarray(be, np.float32)[0]),
        "rep": rep,
        "p32": p32.astype(BF),

        "bst": np.ascontiguousarray(np.asarray(bs, np.float32)[0].T),

        "w4": w4.astype(BF),
        "biast": biast,
        "identb": np.eye(128, dtype=np.float32).astype(BF),
        "sel4": np.vstack([np.vstack([np.eye(T, dtype=np.float32),
                                      np.zeros((8, T), np.float32)])
                           for _ in range(4)]),
    }


TRACE = False
LAST = {}


def kernel(x, edge_index, edge_weight, Ve, be, Vs, bs, cheb_W, cheb_b, res_W, res_b):
    import ml_dtypes
    from concourse.bass_utils import run_bass_kernel_spmd
    BF = ml_dtypes.bfloat16

    x = np.asarray(x, np.float32)
    shared = _host_prep(x, edge_index, edge_weight, Ve, be, Vs, bs,
                        cheb_W, cheb_b, res_W, res_b)
    nc = _build()
    in_maps = []
    for b in range(B):
        m = dict(shared)
        m["xb"] = np.ascontiguousarray(x[b].reshape(N, D)).astype(BF)
        # xbt: row d' = t*32+f  ->  x[b][n, f, t];  (D, N)
        m["xbt"] = np.ascontiguousarray(
            x[b].transpose(2, 1, 0).reshape(D, N)).astype(BF)
        in_maps.append(m)
    res = run_bass_kernel_spmd(nc, in_maps, list(range(B)), trace=TRACE)
    LAST["res"] = res
    # out (1536, 1024): row = c*128 + a*64 + g, t = 2c+a
    out = np.stack(
        [np.asarray(r["out"], np.float32).reshape(12, 2, G, N)
         .transpose(3, 2, 0, 1).reshape(N, G, T)
         for r in res.results], axis=0)
    return out
